# revision 4
# baseline (speedup 1.0000x reference)
"""Trainium2 Bass kernel for the global-context-fusion block.

Reference computation (per batch sample b):
    pooled[c] = mean_{h,w} x[b,c,h,w]                         # [C]
    y1 = relu6(w_guide @ pooled)                              # [R]
    y2 = relu6((w_fuse @ y1 - bn_mean) * inv_std * g + beta)  # [C]
    out[b,c,h,w] = x[b,c,h,w] + y2[c]

Strategy: data-parallel over batch — 8 samples, 8 NeuronCores, one sample per
core; the tiny 1x1-path params are replicated. The kernel is pure HBM traffic:
x must be fully read (pool + residual add) and the output fully written.

To cut traffic, x is cast to fp16 on the host and streamed in at half size
(16 MiB/core), kept entirely resident in SBUF between the pooling pass and the
broadcast-add, and the output is written back as fp16 (16 MiB/core) and upcast
to fp32 on the host. Total DMA traffic is 32 MiB/core instead of the fp32
two-pass 80 MiB/core; both passes run at the ~435 GB/s SBUF-AXI line rate.
The fp16 rounding of x and of (x + y2) contributes a relative error of ~3e-4,
far inside the 2e-2 gate; pooled sums are accumulated in fp32.

Engine schedule (from trace analysis):
  - All bulk elementwise work runs on DVE in its 4x packed mode (16-bit,
    unit-stride): row-sums as identity tensor_scalar with fp32 accum_out
    (1.29 us/MiB-tile; TensorReduce has no packed mode and takes 4.42 us),
    and the broadcast-adds as tensor_scalar_add (1.28 us). DVE stays well
    under the DMA stream rate (~2.4 us/MiB), so both passes are DMA-bound.
  - x loads and out stores ride the SP HWDGE ring back-to-back; the tiny
    param loads ride the otherwise-idle ACT ring so they don't delay the
    first x tile.
  - First-stored / last-loaded tiles are narrowed to shrink the serial
    pool->MLP->first-add transition between the two DMA phases.

Host-side folding (all on tiny [C]-sized tensors):
    wg = (w_guide / HW).T          -> pool division folded into first matmul
    wf = (w_fuse * bn_scale).T     -> BN scale folded into second matmul
    b2 = beta - mean * bn_scale    -> BN shift applied as bias before relu6
"""

import numpy as np

from concourse import bass, mybir, tile
from concourse.bass_utils import run_bass_kernel_spmd

# Problem shapes (nn_GCF_FPGA_68032281969033), hardcoded per harness contract.
B, C, H, W = 8, 512, 128, 128
HW = H * W
R = 128
P = 128
BN_EPS = 1e-5

M_CHUNKS = C // P        # channel chunks of 128 partitions
# Tile widths per chunk. Chunk 0 leads with a narrow tile so the first store
# issues quickly after y2; chunk 3 trails with a narrow tile so the final
# row-sum on the critical path is short.
WIDTHS = [
    [1024, 3072, 4096, 4096, 4096],
    [4096, 4096, 4096, 4096],
    [4096, 4096, 4096, 4096],
    [4096, 4096, 4096, 3072, 1024],
]
assert all(sum(ws) == HW for ws in WIDTHS)
N_TILES = sum(len(ws) for ws in WIDTHS)

FP32 = mybir.dt.float32
FP16 = mybir.dt.float16
AX = mybir.AxisListType.X
ALU = mybir.AluOpType


def _build_program() -> bass.Bass:
    nc = bass.Bass()
    x_d = nc.declare_dram_parameter("x", [C, HW], FP16, isOutput=False)
    wg_d = nc.declare_dram_parameter("wg", [C, R], FP32, isOutput=False)
    wf_d = nc.declare_dram_parameter("wf", [R, C], FP32, isOutput=False)
    # b2 padded to 512 B lines per partition: sub-512 B DMA lines pay the SDMA
    # read-modify-write penalty and stall the ring head.
    b2_d = nc.declare_dram_parameter("b2", [P, 128], FP32, isOutput=False)
    out_d = nc.declare_dram_parameter("out", [C, HW], FP16, isOutput=True)

    with tile.TileContext(nc) as tc:
        with (
            tc.tile_pool(name="params", bufs=1) as ppool,
            tc.tile_pool(name="cache", bufs=1) as cpool,
            tc.tile_pool(name="psum", bufs=1, space="PSUM") as qpool,
        ):
            # Params on the ACT HWDGE ring (no compute is queued there), so
            # the SP ring starts streaming x immediately.
            wg_raw = ppool.tile([P, M_CHUNKS, R], FP32, tag="wg_raw")
            nc.scalar.dma_start(out=wg_raw[:], in_=wg_d.rearrange("(k p) r -> p k r", p=P))
            wf_raw = ppool.tile([P, C], FP32, tag="wf_raw")
            nc.scalar.dma_start(out=wf_raw[:], in_=wf_d[:])
            b2_t = ppool.tile([P, 128], FP32, tag="b2")
            nc.scalar.dma_start(out=b2_t[:], in_=b2_d[:])

            # Matmul (LDWEIGHTS) instructions only get one sync-wait slot in
            # walrus codegen, but they read both DMA-landed weights and
            # DVE-produced activations. Staging the weights through a DVE copy
            # makes every matmul input DVE-produced -> a single DVE wait.
            wg_t = ppool.tile([P, M_CHUNKS, R], FP32, tag="wg")
            nc.vector.tensor_copy(out=wg_t[:], in_=wg_raw[:])
            wf_t = ppool.tile([P, C], FP32, tag="wf")
            nc.vector.tensor_copy(out=wf_t[:], in_=wf_raw[:])

            part_t = ppool.tile([P, N_TILES], FP32, tag="part")
            sums_t = ppool.tile([P, M_CHUNKS], FP32, tag="sums")
            y1_t = ppool.tile([P, 1], FP32, tag="y1")
            y2_t = ppool.tile([P, M_CHUNKS], FP32, tag="y2")

            # Pass 1: stream fp16 x in; row-sum each tile on DVE as an
            # identity tensor_scalar with fp32 FSA accumulation — unlike
            # TensorReduce this op has a 4x packed mode, so it chases the
            # DMA stream with ~1.3 us/MiB to spare.
            cached = {}          # (m, j) -> resident [P, w] fp16 tile
            p1 = qpool.tile([P, 1], FP32, tag="p1")
            col = 0
            chunk_cols = []
            for m in range(M_CHUNKS):
                lo = col
                off = 0
                for j, w in enumerate(WIDTHS[m]):
                    t = cpool.tile([P, w], FP16, tag=f"c{m}_{j}")
                    cached[(m, j)] = t
                    nc.sync.dma_start(
                        out=t[:], in_=x_d[m * P : (m + 1) * P, off : off + w]
                    )
                    # walrus requires both ALU ops on the accumulating
                    # (TensorScalarPtrReduce) form; (x+0)+0 is an identity.
                    nc.vector.tensor_scalar(
                        out=t[:], in0=t[:], scalar1=0.0, scalar2=0.0,
                        op0=ALU.add, op1=ALU.add,
                        accum_out=part_t[:, col : col + 1],
                    )
                    off += w
                    col += 1
                chunk_cols.append((lo, col - lo))
                # Chunk m fully reduced -> combine partials and fold into the
                # y1 matmul accumulation immediately; only chunk 3's combine
                # and matmul remain on the critical path after the last tile.
                nc.vector.reduce_sum(
                    out=sums_t[:, m : m + 1],
                    in_=part_t[:, lo:col],
                    axis=AX,
                )
                nc.tensor.matmul(
                    p1[:],
                    wg_t[:, m, :],
                    sums_t[:, m : m + 1],
                    start=(m == 0),
                    stop=(m == M_CHUNKS - 1),
                )

            # y1 = relu6(p1)
            nc.vector.tensor_scalar(
                out=y1_t[:], in0=p1[:], scalar1=0.0, scalar2=6.0, op0=ALU.max, op1=ALU.min
            )

            # y2 = relu6(wf.T @ y1 + b2): one [128,1] column per channel chunk.
            p2 = qpool.tile([P, M_CHUNKS], FP32, tag="p2")
            for m in range(M_CHUNKS):
                nc.tensor.matmul(
                    p2[:, m : m + 1],
                    wf_t[:, m * P : (m + 1) * P],
                    y1_t[:],
                    start=True,
                    stop=True,
                )
            nc.vector.tensor_add(out=y2_t[:], in0=p2[:], in1=b2_t[:, :M_CHUNKS])
            nc.vector.tensor_scalar(
                out=y2_t[:], in0=y2_t[:], scalar1=0.0, scalar2=6.0, op0=ALU.max, op1=ALU.min
            )

            # Pass 2: out = x + y2[channel], in place on the resident fp16
            # tiles (DVE 4x mode), then store on the SP ring. DVE outpaces the
            # store drain, so this pass is DMA-bound end to end.
            for m in range(M_CHUNKS):
                off = 0
                for j, w in enumerate(WIDTHS[m]):
                    t = cached[(m, j)]
                    nc.vector.tensor_scalar_add(
                        out=t[:], in0=t[:], scalar1=y2_t[:, m : m + 1]
                    )
                    nc.sync.dma_start(
                        out=out_d[m * P : (m + 1) * P, off : off + w], in_=t[:]
                    )
                    off += w

    _hoist_excess_waits(nc)
    return nc


# walrus codegen has per-instruction sync-wait slot limits (the Matmult
# LDWEIGHTS struct fits one wait; the DMA DIRECT2D struct fits two). Tile's
# sem assignment is not transitively minimal and can exceed them. Excess waits
# are hoisted into standalone EventSemaphore instructions placed right before
# the instruction on the same engine queue — identical semantics (inline DMA
# waits execute at the issuing sequencer too), just a different encoding.
_WAIT_CAPS = {
    "InstMatmult": 1,
    "InstActivation": 1,
    "InstDMACopy": 1,
    "InstTensorReduce": 1,
    "InstTensorScalarPtr": 1,
    "InstTensorTensor": 1,
    "InstTensorCopy": 1,
    "InstMemset": 1,
    "InstDrain": 1,
}


def _hoist_excess_waits(nc: bass.Bass) -> None:
    n = 0
    for bb in nc.main_func.blocks:
        il = bb.instructions
        new_list = []
        for ins in il:
            si = ins.sync_info
            cap = _WAIT_CAPS.get(type(ins).__name__)
            if si is not None and cap is not None and len(si.on_wait) > cap:
                waits = list(si.on_wait)
                for w in waits[cap:]:
                    n += 1
                    es = mybir.InstEventSemaphore(
                        name=f"I-hoistwait-{n}",
                        engine=ins.engine,
                        sync_info=mybir.SyncInfo(on_wait=[w], on_update=[]),
                    )
                    new_list.append(es)
                ins.sync_info = mybir.SyncInfo(
                    on_wait=waits[:cap], on_update=list(si.on_update)
                )
            new_list.append(ins)
        if len(new_list) != len(il):
            il[:] = new_list


_NC = None


def _get_nc() -> bass.Bass:
    global _NC
    if _NC is None:
        _NC = _build_program()
    return _NC


def _prep_in_maps(x, w_guide, w_fuse, bn_gamma, bn_beta, bn_mean, bn_var):
    x = np.asarray(x, dtype=np.float32)
    w_guide = np.asarray(w_guide, dtype=np.float32)
    w_fuse = np.asarray(w_fuse, dtype=np.float32)
    bn_gamma = np.asarray(bn_gamma, dtype=np.float32)
    bn_beta = np.asarray(bn_beta, dtype=np.float32)
    bn_mean = np.asarray(bn_mean, dtype=np.float32)
    bn_var = np.asarray(bn_var, dtype=np.float32)

    scale = bn_gamma / np.sqrt(bn_var + np.float32(BN_EPS))
    wg = np.ascontiguousarray((w_guide / np.float32(HW)).T)           # [C, R]
    wf = np.ascontiguousarray((w_fuse * scale[:, None]).T)            # [R, C]
    b2 = np.zeros((P, 128), dtype=np.float32)  # padded to 512 B DMA lines
    b2[:, :M_CHUNKS] = (bn_beta - bn_mean * scale).reshape(M_CHUNKS, P).T

    xs = np.ascontiguousarray(x.reshape(B, C, HW).astype(np.float16))
    return [{"x": xs[i], "wg": wg, "wf": wf, "b2": b2} for i in range(B)]


def run(inputs: dict, **kwargs):
    """Run the SPMD kernel; returns the BassKernelResults (for profiling)."""
    nc = _get_nc()
    in_maps = _prep_in_maps(**inputs)
    return run_bass_kernel_spmd(nc, in_maps, core_ids=list(range(B)), **kwargs)


def kernel(**inputs) -> np.ndarray:
    res = run(inputs)
    out = np.stack([np.asarray(res.results[i]["out"]) for i in range(B)], axis=0)
    return out.reshape(B, C, H, W).astype(np.float32)


# revision 6
# speedup vs baseline: 1.2453x; 1.2453x over previous
"""Trainium2 Bass kernel for the global-context-fusion block.

Reference computation (per batch sample b):
    pooled[c] = mean_{h,w} x[b,c,h,w]                         # [C]
    y1 = relu6(w_guide @ pooled)                              # [R]
    y2 = relu6((w_fuse @ y1 - bn_mean) * inv_std * g + beta)  # [C]
    out[b,c,h,w] = x[b,c,h,w] + y2[c]

Strategy: data-parallel over batch — 8 samples, 8 NeuronCores, one sample per
core; the tiny 1x1-path params are replicated. The kernel is pure HBM traffic:
x must be fully read (pool + residual add) and the output fully written.

To cut traffic, x is cast to fp16 on the host and streamed in at half size
(16 MiB/core), kept entirely resident in SBUF between the pooling pass and the
broadcast-add, and the output is written back as fp16 (16 MiB/core) and upcast
to fp32 on the host. Total DMA traffic is 32 MiB/core instead of the fp32
two-pass 80 MiB/core; both passes run at the ~435 GB/s SBUF-AXI line rate.
The fp16 rounding of x and of (x + y2) contributes a relative error of ~3e-4,
far inside the 2e-2 gate; pooled sums are accumulated in fp32.

Engine schedule (from trace analysis):
  - All bulk elementwise work runs on DVE in its 4x packed mode (16-bit,
    unit-stride): row-sums as identity tensor_scalar with fp32 accum_out
    (1.29 us/MiB-tile; TensorReduce has no packed mode and takes 4.42 us),
    and the broadcast-adds as tensor_scalar_add (1.28 us). DVE stays well
    under the DMA stream rate (~2.4 us/MiB), so both passes are DMA-bound.
  - x loads and out stores ride the SP HWDGE ring back-to-back; the tiny
    param loads ride the otherwise-idle ACT ring so they don't delay the
    first x tile.
  - First-stored / last-loaded tiles are narrowed to shrink the serial
    pool->MLP->first-add transition between the two DMA phases.

Host-side folding (all on tiny [C]-sized tensors):
    wg = (w_guide / HW).T          -> pool division folded into first matmul
    wf = (w_fuse * bn_scale).T     -> BN scale folded into second matmul
    b2 = beta - mean * bn_scale    -> BN shift applied as bias before relu6
"""

import numpy as np

from concourse import bass, mybir, tile
from concourse.bass_utils import run_bass_kernel_spmd

# Problem shapes (nn_GCF_FPGA_68032281969033), hardcoded per harness contract.
B, C, H, W = 8, 512, 128, 128
HW = H * W
R = 128
P = 128
BN_EPS = 1e-5

M_CHUNKS = C // P        # channel chunks of 128 partitions
# Tile widths per chunk. Chunk 0 leads with a narrow tile so the first store
# issues quickly after y2; chunk 3 trails with a narrow tile so the final
# row-sum on the critical path is short.
WIDTHS = [
    [1024, 3072, 4096, 4096, 4096],
    [4096, 4096, 4096, 4096],
    [4096, 4096, 4096, 4096],
    [4096, 4096, 4096, 3072, 1024],
]
assert all(sum(ws) == HW for ws in WIDTHS)
N_TILES = sum(len(ws) for ws in WIDTHS)

FP32 = mybir.dt.float32
FP16 = mybir.dt.float16
AX = mybir.AxisListType.X
ALU = mybir.AluOpType


def _build_program() -> bass.Bass:
    nc = bass.Bass()
    x_d = nc.declare_dram_parameter("x", [C, HW], FP16, isOutput=False)
    wg_d = nc.declare_dram_parameter("wg", [C, R], FP32, isOutput=False)
    wf_d = nc.declare_dram_parameter("wf", [R, C], FP32, isOutput=False)
    # b2 padded to 512 B lines per partition: sub-512 B DMA lines pay the SDMA
    # read-modify-write penalty and stall the ring head.
    b2_d = nc.declare_dram_parameter("b2", [P, 128], FP32, isOutput=False)
    out_d = nc.declare_dram_parameter("out", [C, HW], FP16, isOutput=True)

    with tile.TileContext(nc) as tc:
        with (
            tc.tile_pool(name="params", bufs=1) as ppool,
            tc.tile_pool(name="cache", bufs=1) as cpool,
            tc.tile_pool(name="psum", bufs=1, space="PSUM") as qpool,
        ):
            # Params on the ACT HWDGE ring (no compute is queued there), so
            # the SP ring starts streaming x immediately.
            wg_raw = ppool.tile([P, M_CHUNKS, R], FP32, tag="wg_raw")
            nc.scalar.dma_start(out=wg_raw[:], in_=wg_d.rearrange("(k p) r -> p k r", p=P))
            wf_raw = ppool.tile([P, C], FP32, tag="wf_raw")
            nc.scalar.dma_start(out=wf_raw[:], in_=wf_d[:])
            b2_t = ppool.tile([P, 128], FP32, tag="b2")
            nc.scalar.dma_start(out=b2_t[:], in_=b2_d[:])

            # Matmul (LDWEIGHTS) instructions only get one sync-wait slot in
            # walrus codegen, but they read both DMA-landed weights and
            # DVE-produced activations. Staging the weights through a DVE copy
            # makes every matmul input DVE-produced -> a single DVE wait.
            wg_t = ppool.tile([P, M_CHUNKS, R], FP32, tag="wg")
            nc.vector.tensor_copy(out=wg_t[:], in_=wg_raw[:])
            wf_t = ppool.tile([P, C], FP32, tag="wf")
            nc.vector.tensor_copy(out=wf_t[:], in_=wf_raw[:])

            part_t = ppool.tile([P, N_TILES], FP32, tag="part")
            sums_t = ppool.tile([P, M_CHUNKS], FP32, tag="sums")
            y1_t = ppool.tile([P, 1], FP32, tag="y1")
            y2_t = ppool.tile([P, M_CHUNKS], FP32, tag="y2")

            # Pass 1: stream fp16 x in; row-sum each tile as it lands. All
            # reduction forms run at 1 elem/cycle/partition (TensorReduce has
            # no packed mode, and the accumulating TensorScalarPtr falls back
            # to 1x on HW despite the cost model's claim; GPSIMD supports
            # neither free-axis reduce nor the accum form), so the work is
            # split between ACT (copy with fp32 FSA accum, 1.2 GHz) and DVE
            # (reduce_sum, 0.96 GHz) with a greedy arrival-aware schedule —
            # combined 0.50 tile/us vs the 0.42 tile/us DMA stream.
            def row_sum(t, col, eng):
                if eng == "D":
                    nc.vector.reduce_sum(
                        out=part_t[:, col : col + 1], in_=t[:], axis=AX
                    )
                else:
                    nc.scalar.activation(
                        out=t[:],
                        in_=t[:],
                        func=mybir.ActivationFunctionType.Copy,
                        accum_out=part_t[:, col : col + 1],
                    )

            # Greedy engine choice simulated against the DMA arrival clock
            # (2.4 us/MiB): pick the engine that can start each tile soonest.
            flat_w = [w for ws in WIDTHS for w in ws]
            arrive = []
            tclk = 0.0
            for w in flat_w:
                tclk += 2.4 * (w / 4096.0)
                arrive.append(tclk)
            ready = {"A": 4.0, "D": 2.0}   # ACT table-load + param issue lag
            DUR = {"A": 3.71 / 4096.0, "D": 4.42 / 4096.0}
            engs = []
            for w, ar in zip(flat_w, arrive):
                starts = {e: max(ar, ready[e]) for e in ("A", "D")}
                e = min(starts, key=lambda e: (starts[e] + DUR[e] * w))
                ready[e] = starts[e] + DUR[e] * w
                engs.append(e)

            cached = {}          # (m, j) -> resident [P, w] fp16 tile
            p1 = qpool.tile([P, 1], FP32, tag="p1")
            col = 0
            for m in range(M_CHUNKS):
                lo = col
                off = 0
                for j, w in enumerate(WIDTHS[m]):
                    t = cpool.tile([P, w], FP16, tag=f"c{m}_{j}")
                    cached[(m, j)] = t
                    nc.sync.dma_start(
                        out=t[:], in_=x_d[m * P : (m + 1) * P, off : off + w]
                    )
                    row_sum(t, col, engs[col])
                    off += w
                    col += 1
                # Chunk m fully reduced -> combine partials and fold into the
                # y1 matmul accumulation immediately; only chunk 3's combine
                # and matmul remain on the critical path after the last tile.
                nc.vector.reduce_sum(
                    out=sums_t[:, m : m + 1],
                    in_=part_t[:, lo:col],
                    axis=AX,
                )
                nc.tensor.matmul(
                    p1[:],
                    wg_t[:, m, :],
                    sums_t[:, m : m + 1],
                    start=(m == 0),
                    stop=(m == M_CHUNKS - 1),
                )

            # y1 = relu6(p1)
            nc.vector.tensor_scalar(
                out=y1_t[:], in0=p1[:], scalar1=0.0, scalar2=6.0, op0=ALU.max, op1=ALU.min
            )

            # y2 = relu6(wf.T @ y1 + b2): one [128,1] column per channel chunk.
            p2 = qpool.tile([P, M_CHUNKS], FP32, tag="p2")
            for m in range(M_CHUNKS):
                nc.tensor.matmul(
                    p2[:, m : m + 1],
                    wf_t[:, m * P : (m + 1) * P],
                    y1_t[:],
                    start=True,
                    stop=True,
                )
            nc.vector.tensor_add(out=y2_t[:], in0=p2[:], in1=b2_t[:, :M_CHUNKS])
            nc.vector.tensor_scalar(
                out=y2_t[:], in0=y2_t[:], scalar1=0.0, scalar2=6.0, op0=ALU.max, op1=ALU.min
            )

            # Pass 2: out = x + y2[channel], in place on the resident fp16
            # tiles (DVE 4x mode), then store on the SP ring. DVE outpaces the
            # store drain, so this pass is DMA-bound end to end.
            for m in range(M_CHUNKS):
                off = 0
                for j, w in enumerate(WIDTHS[m]):
                    t = cached[(m, j)]
                    nc.vector.tensor_scalar_add(
                        out=t[:], in0=t[:], scalar1=y2_t[:, m : m + 1]
                    )
                    nc.sync.dma_start(
                        out=out_d[m * P : (m + 1) * P, off : off + w], in_=t[:]
                    )
                    off += w

    _hoist_excess_waits(nc)
    return nc


# walrus codegen has per-instruction sync-wait slot limits (the Matmult
# LDWEIGHTS struct fits one wait; the DMA DIRECT2D struct fits two). Tile's
# sem assignment is not transitively minimal and can exceed them. Excess waits
# are hoisted into standalone EventSemaphore instructions placed right before
# the instruction on the same engine queue — identical semantics (inline DMA
# waits execute at the issuing sequencer too), just a different encoding.
_WAIT_CAPS = {
    "InstMatmult": 1,
    "InstActivation": 1,
    "InstDMACopy": 1,
    "InstTensorReduce": 1,
    "InstTensorScalarPtr": 1,
    "InstTensorTensor": 1,
    "InstTensorCopy": 1,
    "InstMemset": 1,
    "InstDrain": 1,
}


def _hoist_excess_waits(nc: bass.Bass) -> None:
    n = 0
    for bb in nc.main_func.blocks:
        il = bb.instructions
        new_list = []
        for ins in il:
            si = ins.sync_info
            cap = _WAIT_CAPS.get(type(ins).__name__)
            if si is not None and cap is not None and len(si.on_wait) > cap:
                waits = list(si.on_wait)
                for w in waits[cap:]:
                    n += 1
                    es = mybir.InstEventSemaphore(
                        name=f"I-hoistwait-{n}",
                        engine=ins.engine,
                        sync_info=mybir.SyncInfo(on_wait=[w], on_update=[]),
                    )
                    new_list.append(es)
                ins.sync_info = mybir.SyncInfo(
                    on_wait=waits[:cap], on_update=list(si.on_update)
                )
            new_list.append(ins)
        if len(new_list) != len(il):
            il[:] = new_list


_NC = None


def _get_nc() -> bass.Bass:
    global _NC
    if _NC is None:
        _NC = _build_program()
    return _NC


def _prep_in_maps(x, w_guide, w_fuse, bn_gamma, bn_beta, bn_mean, bn_var):
    x = np.asarray(x, dtype=np.float32)
    w_guide = np.asarray(w_guide, dtype=np.float32)
    w_fuse = np.asarray(w_fuse, dtype=np.float32)
    bn_gamma = np.asarray(bn_gamma, dtype=np.float32)
    bn_beta = np.asarray(bn_beta, dtype=np.float32)
    bn_mean = np.asarray(bn_mean, dtype=np.float32)
    bn_var = np.asarray(bn_var, dtype=np.float32)

    scale = bn_gamma / np.sqrt(bn_var + np.float32(BN_EPS))
    wg = np.ascontiguousarray((w_guide / np.float32(HW)).T)           # [C, R]
    wf = np.ascontiguousarray((w_fuse * scale[:, None]).T)            # [R, C]
    b2 = np.zeros((P, 128), dtype=np.float32)  # padded to 512 B DMA lines
    b2[:, :M_CHUNKS] = (bn_beta - bn_mean * scale).reshape(M_CHUNKS, P).T

    xs = np.ascontiguousarray(x.reshape(B, C, HW).astype(np.float16))
    return [{"x": xs[i], "wg": wg, "wf": wf, "b2": b2} for i in range(B)]


def run(inputs: dict, **kwargs):
    """Run the SPMD kernel; returns the BassKernelResults (for profiling)."""
    nc = _get_nc()
    in_maps = _prep_in_maps(**inputs)
    return run_bass_kernel_spmd(nc, in_maps, core_ids=list(range(B)), **kwargs)


def kernel(**inputs) -> np.ndarray:
    res = run(inputs)
    out = np.stack([np.asarray(res.results[i]["out"]) for i in range(B)], axis=0)
    return out.reshape(B, C, H, W).astype(np.float32)


# revision 8
# speedup vs baseline: 1.3728x; 1.1024x over previous
"""Trainium2 Bass kernel for the global-context-fusion block.

Reference computation (per batch sample b):
    pooled[c] = mean_{h,w} x[b,c,h,w]                         # [C]
    y1 = relu6(w_guide @ pooled)                              # [R]
    y2 = relu6((w_fuse @ y1 - bn_mean) * inv_std * g + beta)  # [C]
    out[b,c,h,w] = x[b,c,h,w] + y2[c]

Strategy: data-parallel over batch — 8 samples, 8 NeuronCores, one sample per
core; the tiny 1x1-path params are replicated. The kernel is pure HBM traffic:
x must be fully read (pool + residual add) and the output fully written.

To cut traffic, x is cast to fp16 on the host and streamed in at half size
(16 MiB/core), kept entirely resident in SBUF between the pooling pass and the
broadcast-add, and the output is written back as fp16 (16 MiB/core) and upcast
to fp32 on the host. Total DMA traffic is 32 MiB/core instead of the fp32
two-pass 80 MiB/core; both passes run at the ~435 GB/s SBUF-AXI line rate.
The fp16 rounding of x and of (x + y2) contributes a relative error of ~3e-4,
far inside the 2e-2 gate; pooled sums are accumulated in fp32.

Engine schedule (from trace analysis):
  - All bulk elementwise work runs on DVE in its 4x packed mode (16-bit,
    unit-stride): row-sums as identity tensor_scalar with fp32 accum_out
    (1.29 us/MiB-tile; TensorReduce has no packed mode and takes 4.42 us),
    and the broadcast-adds as tensor_scalar_add (1.28 us). DVE stays well
    under the DMA stream rate (~2.4 us/MiB), so both passes are DMA-bound.
  - x loads and out stores ride the SP HWDGE ring back-to-back; the tiny
    param loads ride the otherwise-idle ACT ring so they don't delay the
    first x tile.
  - First-stored / last-loaded tiles are narrowed to shrink the serial
    pool->MLP->first-add transition between the two DMA phases.

Host-side folding (all on tiny [C]-sized tensors):
    wg = (w_guide / HW).T          -> pool division folded into first matmul
    wf = (w_fuse * bn_scale).T     -> BN scale folded into second matmul
    b2 = beta - mean * bn_scale    -> BN shift applied as bias before relu6
"""

import numpy as np

from concourse import bass, mybir, tile
from concourse.bass_utils import run_bass_kernel_spmd

# Problem shapes (nn_GCF_FPGA_68032281969033), hardcoded per harness contract.
B, C, H, W = 8, 512, 128, 128
HW = H * W
R = 128
P = 128
BN_EPS = 1e-5

M_CHUNKS = C // P        # channel chunks of 128 partitions
# Tile widths per chunk. Chunk 0 leads with a narrow tile so the first store
# issues quickly after y2; chunk 3 trails with a narrow tile so the final
# row-sum on the critical path is short.
WIDTHS = [
    [1024, 3072, 4096, 4096, 4096],
    [4096, 4096, 4096, 4096],
    [4096, 4096, 4096, 4096],
    [4096, 4096, 4096, 3072, 1024],
]
assert all(sum(ws) == HW for ws in WIDTHS)
N_TILES = sum(len(ws) for ws in WIDTHS)

FP32 = mybir.dt.float32
FP16 = mybir.dt.float16
AX = mybir.AxisListType.X
ALU = mybir.AluOpType


def _build_program() -> bass.Bass:
    nc = bass.Bass()
    x_d = nc.declare_dram_parameter("x", [C, HW], FP16, isOutput=False)
    wg_d = nc.declare_dram_parameter("wg", [C, R], FP32, isOutput=False)
    wf_d = nc.declare_dram_parameter("wf", [R, C], FP32, isOutput=False)
    # b2 padded to 512 B lines per partition: sub-512 B DMA lines pay the SDMA
    # read-modify-write penalty and stall the ring head.
    b2_d = nc.declare_dram_parameter("b2", [P, 128], FP32, isOutput=False)
    out_d = nc.declare_dram_parameter("out", [C, HW], FP16, isOutput=True)

    with tile.TileContext(nc) as tc:
        with (
            tc.tile_pool(name="params", bufs=1) as ppool,
            tc.tile_pool(name="cache", bufs=1) as cpool,
            tc.tile_pool(name="psum", bufs=1, space="PSUM") as qpool,
        ):
            # Params at the head of the SP ring: they are small and drain in a
            # couple of microseconds before the bulk x-loads start. (Putting
            # them on the ACT ring stalls later x-loads on DMA sem-lane
            # recycling: the param lanes are not cleared until the late
            # staging copies run, and the SDMA stream goes idle — measured
            # +9 us on pass 1.)
            wg_raw = ppool.tile([P, M_CHUNKS, R], FP32, tag="wg_raw")
            nc.sync.dma_start(out=wg_raw[:], in_=wg_d.rearrange("(k p) r -> p k r", p=P))
            wf_raw = ppool.tile([P, C], FP32, tag="wf_raw")
            nc.sync.dma_start(out=wf_raw[:], in_=wf_d[:])
            b2_t = ppool.tile([P, 128], FP32, tag="b2")
            nc.sync.dma_start(out=b2_t[:], in_=b2_d[:])

            # Matmul (LDWEIGHTS) instructions only get one sync-wait slot in
            # walrus codegen, but they read both DMA-landed weights and
            # DVE-produced activations. Staging the weights through a DVE copy
            # makes every matmul input DVE-produced -> a single DVE wait.
            wg_t = ppool.tile([P, M_CHUNKS, R], FP32, tag="wg")
            nc.vector.tensor_copy(out=wg_t[:], in_=wg_raw[:])
            wf_t = ppool.tile([P, C], FP32, tag="wf")
            nc.vector.tensor_copy(out=wf_t[:], in_=wf_raw[:])

            part_t = ppool.tile([P, N_TILES], FP32, tag="part")
            sums_t = ppool.tile([P, M_CHUNKS], FP32, tag="sums")
            y1_t = ppool.tile([P, 1], FP32, tag="y1")
            y2_t = ppool.tile([P, M_CHUNKS], FP32, tag="y2")

            # Pass 1: stream fp16 x in; row-sum each tile as it lands. All
            # reduction forms run at 1 elem/cycle/partition (TensorReduce has
            # no packed mode, and the accumulating TensorScalarPtr falls back
            # to 1x on HW despite the cost model's claim; GPSIMD supports
            # neither free-axis reduce nor the accum form), so the work is
            # split between ACT (copy with fp32 FSA accum, 1.2 GHz) and DVE
            # (reduce_sum, 0.96 GHz) with a greedy arrival-aware schedule —
            # combined 0.50 tile/us vs the 0.42 tile/us DMA stream.
            def row_sum(t, col, eng):
                if eng == "D":
                    nc.vector.reduce_sum(
                        out=part_t[:, col : col + 1], in_=t[:], axis=AX
                    )
                else:
                    nc.scalar.activation(
                        out=t[:],
                        in_=t[:],
                        func=mybir.ActivationFunctionType.Copy,
                        accum_out=part_t[:, col : col + 1],
                    )

            # Greedy engine choice simulated against the DMA arrival clock
            # (2.4 us/MiB): pick the engine that can start each tile soonest.
            flat_w = [w for ws in WIDTHS for w in ws]
            arrive = []
            tclk = 0.0
            for w in flat_w:
                tclk += 2.4 * (w / 4096.0)
                arrive.append(tclk)
            ready = {"A": 2.0, "D": 3.5}   # ACT table-load; DVE staging copies
            DUR = {"A": 3.71 / 4096.0, "D": 4.42 / 4096.0}
            engs = []
            for w, ar in zip(flat_w, arrive):
                starts = {e: max(ar, ready[e]) for e in ("A", "D")}
                e = min(starts, key=lambda e: (starts[e] + DUR[e] * w))
                ready[e] = starts[e] + DUR[e] * w
                engs.append(e)

            cached = {}          # (m, j) -> resident [P, w] fp16 tile
            p1 = qpool.tile([P, 1], FP32, tag="p1")
            col = 0
            for m in range(M_CHUNKS):
                lo = col
                off = 0
                for j, w in enumerate(WIDTHS[m]):
                    t = cpool.tile([P, w], FP16, tag=f"c{m}_{j}")
                    cached[(m, j)] = t
                    nc.sync.dma_start(
                        out=t[:], in_=x_d[m * P : (m + 1) * P, off : off + w]
                    )
                    row_sum(t, col, engs[col])
                    off += w
                    col += 1
                # Chunk m fully reduced -> combine partials and fold into the
                # y1 matmul accumulation immediately; only chunk 3's combine
                # and matmul remain on the critical path after the last tile.
                nc.vector.reduce_sum(
                    out=sums_t[:, m : m + 1],
                    in_=part_t[:, lo:col],
                    axis=AX,
                )
                nc.tensor.matmul(
                    p1[:],
                    wg_t[:, m, :],
                    sums_t[:, m : m + 1],
                    start=(m == 0),
                    stop=(m == M_CHUNKS - 1),
                )

            # y1 = relu6(p1)
            nc.vector.tensor_scalar(
                out=y1_t[:], in0=p1[:], scalar1=0.0, scalar2=6.0, op0=ALU.max, op1=ALU.min
            )

            # y2 = relu6(wf.T @ y1 + b2): one [128,1] column per channel chunk.
            p2 = qpool.tile([P, M_CHUNKS], FP32, tag="p2")
            for m in range(M_CHUNKS):
                nc.tensor.matmul(
                    p2[:, m : m + 1],
                    wf_t[:, m * P : (m + 1) * P],
                    y1_t[:],
                    start=True,
                    stop=True,
                )
            nc.vector.tensor_add(out=y2_t[:], in0=p2[:], in1=b2_t[:, :M_CHUNKS])
            nc.vector.tensor_scalar(
                out=y2_t[:], in0=y2_t[:], scalar1=0.0, scalar2=6.0, op0=ALU.max, op1=ALU.min
            )

            # Pass 2: out = x + y2[channel], in place on the resident fp16
            # tiles (DVE 4x mode), then store on the SP ring. DVE outpaces the
            # store drain, so this pass is DMA-bound end to end.
            for m in range(M_CHUNKS):
                off = 0
                for j, w in enumerate(WIDTHS[m]):
                    t = cached[(m, j)]
                    nc.vector.tensor_scalar_add(
                        out=t[:], in0=t[:], scalar1=y2_t[:, m : m + 1]
                    )
                    nc.sync.dma_start(
                        out=out_d[m * P : (m + 1) * P, off : off + w], in_=t[:]
                    )
                    off += w

    _hoist_excess_waits(nc)
    return nc


# walrus codegen has per-instruction sync-wait slot limits (the Matmult
# LDWEIGHTS struct fits one wait; the DMA DIRECT2D struct fits two). Tile's
# sem assignment is not transitively minimal and can exceed them. Excess waits
# are hoisted into standalone EventSemaphore instructions placed right before
# the instruction on the same engine queue — identical semantics (inline DMA
# waits execute at the issuing sequencer too), just a different encoding.
_WAIT_CAPS = {
    "InstMatmult": 1,
    "InstActivation": 1,
    "InstDMACopy": 1,
    "InstTensorReduce": 1,
    "InstTensorScalarPtr": 1,
    "InstTensorTensor": 1,
    "InstTensorCopy": 1,
    "InstMemset": 1,
    "InstDrain": 1,
}


def _hoist_excess_waits(nc: bass.Bass) -> None:
    n = 0
    for bb in nc.main_func.blocks:
        il = bb.instructions
        new_list = []
        for ins in il:
            si = ins.sync_info
            cap = _WAIT_CAPS.get(type(ins).__name__)
            if si is not None and cap is not None and len(si.on_wait) > cap:
                waits = list(si.on_wait)
                for w in waits[cap:]:
                    n += 1
                    es = mybir.InstEventSemaphore(
                        name=f"I-hoistwait-{n}",
                        engine=ins.engine,
                        sync_info=mybir.SyncInfo(on_wait=[w], on_update=[]),
                    )
                    new_list.append(es)
                ins.sync_info = mybir.SyncInfo(
                    on_wait=waits[:cap], on_update=list(si.on_update)
                )
            new_list.append(ins)
        if len(new_list) != len(il):
            il[:] = new_list


_NC = None


def _get_nc() -> bass.Bass:
    global _NC
    if _NC is None:
        _NC = _build_program()
    return _NC


def _prep_in_maps(x, w_guide, w_fuse, bn_gamma, bn_beta, bn_mean, bn_var):
    x = np.asarray(x, dtype=np.float32)
    w_guide = np.asarray(w_guide, dtype=np.float32)
    w_fuse = np.asarray(w_fuse, dtype=np.float32)
    bn_gamma = np.asarray(bn_gamma, dtype=np.float32)
    bn_beta = np.asarray(bn_beta, dtype=np.float32)
    bn_mean = np.asarray(bn_mean, dtype=np.float32)
    bn_var = np.asarray(bn_var, dtype=np.float32)

    scale = bn_gamma / np.sqrt(bn_var + np.float32(BN_EPS))
    wg = np.ascontiguousarray((w_guide / np.float32(HW)).T)           # [C, R]
    wf = np.ascontiguousarray((w_fuse * scale[:, None]).T)            # [R, C]
    b2 = np.zeros((P, 128), dtype=np.float32)  # padded to 512 B DMA lines
    b2[:, :M_CHUNKS] = (bn_beta - bn_mean * scale).reshape(M_CHUNKS, P).T

    xs = np.ascontiguousarray(x.reshape(B, C, HW).astype(np.float16))
    return [{"x": xs[i], "wg": wg, "wf": wf, "b2": b2} for i in range(B)]


def run(inputs: dict, **kwargs):
    """Run the SPMD kernel; returns the BassKernelResults (for profiling)."""
    nc = _get_nc()
    in_maps = _prep_in_maps(**inputs)
    return run_bass_kernel_spmd(nc, in_maps, core_ids=list(range(B)), **kwargs)


def kernel(**inputs) -> np.ndarray:
    res = run(inputs)
    out = np.stack([np.asarray(res.results[i]["out"]) for i in range(B)], axis=0)
    return out.reshape(B, C, H, W).astype(np.float32)


# revision 10
# speedup vs baseline: 1.4787x; 1.0771x over previous
"""Trainium2 Bass kernel for the global-context-fusion block.

Reference computation (per batch sample b):
    pooled[c] = mean_{h,w} x[b,c,h,w]                         # [C]
    y1 = relu6(w_guide @ pooled)                              # [R]
    y2 = relu6((w_fuse @ y1 - bn_mean) * inv_std * g + beta)  # [C]
    out[b,c,h,w] = x[b,c,h,w] + y2[c]

Strategy: data-parallel over batch — 8 samples, 8 NeuronCores, one sample per
core; the tiny 1x1-path params are replicated. The kernel is pure HBM traffic:
x must be fully read (pool + residual add) and the output fully written.

To cut traffic, x is cast to fp16 on the host and streamed in at half size
(16 MiB/core), kept entirely resident in SBUF between the pooling pass and the
broadcast-add, and the output is written back as fp16 (16 MiB/core) and upcast
to fp32 on the host. Total DMA traffic is 32 MiB/core instead of the fp32
two-pass 80 MiB/core; both passes run at the ~435 GB/s SBUF-AXI line rate.
The fp16 rounding of x and of (x + y2) contributes a relative error of ~3e-4,
far inside the 2e-2 gate; pooled sums are accumulated in fp32.

Engine schedule (from trace analysis):
  - All bulk elementwise work runs on DVE in its 4x packed mode (16-bit,
    unit-stride): row-sums as identity tensor_scalar with fp32 accum_out
    (1.29 us/MiB-tile; TensorReduce has no packed mode and takes 4.42 us),
    and the broadcast-adds as tensor_scalar_add (1.28 us). DVE stays well
    under the DMA stream rate (~2.4 us/MiB), so both passes are DMA-bound.
  - x loads and out stores ride the SP HWDGE ring back-to-back; the tiny
    param loads ride the otherwise-idle ACT ring so they don't delay the
    first x tile.
  - First-stored / last-loaded tiles are narrowed to shrink the serial
    pool->MLP->first-add transition between the two DMA phases.

Host-side folding (all on tiny [C]-sized tensors):
    wg = (w_guide / HW).T          -> pool division folded into first matmul
    wf = (w_fuse * bn_scale).T     -> BN scale folded into second matmul
    b2 = beta - mean * bn_scale    -> BN shift applied as bias before relu6
"""

import numpy as np

from concourse import bass, mybir, tile
from concourse.bass_utils import run_bass_kernel_spmd

# Problem shapes (nn_GCF_FPGA_68032281969033), hardcoded per harness contract.
B, C, H, W = 8, 512, 128, 128
HW = H * W
R = 128
P = 128
BN_EPS = 1e-5

M_CHUNKS = C // P        # channel chunks of 128 partitions
# Tile widths per chunk. Chunk 0 leads with a narrow tile so the first store
# issues quickly after y2; chunk 3 trails with a narrow tile so the final
# row-sum on the critical path is short.
WIDTHS = [
    [1024, 3072, 4096, 4096, 4096],
    [4096, 4096, 4096, 4096],
    [4096, 4096, 4096, 4096],
    [4096, 4096, 2560, 2048, 1536, 1024, 1024],
]
assert all(sum(ws) == HW for ws in WIDTHS)
N_TILES = sum(len(ws) for ws in WIDTHS)

FP32 = mybir.dt.float32
FP16 = mybir.dt.float16
AX = mybir.AxisListType.X
ALU = mybir.AluOpType


def _build_program() -> bass.Bass:
    nc = bass.Bass()
    x_d = nc.declare_dram_parameter("x", [C, HW], FP16, isOutput=False)
    wg_d = nc.declare_dram_parameter("wg", [C, R], FP32, isOutput=False)
    wf_d = nc.declare_dram_parameter("wf", [R, C], FP32, isOutput=False)
    # b2 padded to 512 B lines per partition: sub-512 B DMA lines pay the SDMA
    # read-modify-write penalty and stall the ring head.
    b2_d = nc.declare_dram_parameter("b2", [P, 128], FP32, isOutput=False)
    out_d = nc.declare_dram_parameter("out", [C, HW], FP16, isOutput=True)

    with tile.TileContext(nc) as tc:
        with (
            tc.tile_pool(name="params", bufs=1) as ppool,
            tc.tile_pool(name="cache", bufs=1) as cpool,
            tc.tile_pool(name="psum", bufs=1, space="PSUM") as qpool,
        ):
            # Params at the head of the SP ring: they are small and drain in a
            # couple of microseconds before the bulk x-loads start. (Putting
            # them on the ACT ring stalls later x-loads on DMA sem-lane
            # recycling: the param lanes are not cleared until the late
            # staging copies run, and the SDMA stream goes idle — measured
            # +9 us on pass 1.)
            wg_raw = ppool.tile([P, M_CHUNKS, R], FP32, tag="wg_raw")
            nc.sync.dma_start(out=wg_raw[:], in_=wg_d.rearrange("(k p) r -> p k r", p=P))
            wf_raw = ppool.tile([P, C], FP32, tag="wf_raw")
            nc.sync.dma_start(out=wf_raw[:], in_=wf_d[:])
            b2_t = ppool.tile([P, 128], FP32, tag="b2")
            nc.sync.dma_start(out=b2_t[:], in_=b2_d[:])

            # Matmul (LDWEIGHTS) instructions only get one sync-wait slot in
            # walrus codegen, but they read both DMA-landed weights and
            # DVE-produced activations. Staging the weights through a DVE copy
            # makes every matmul input DVE-produced -> a single DVE wait.
            wg_t = ppool.tile([P, M_CHUNKS, R], FP32, tag="wg")
            nc.vector.tensor_copy(out=wg_t[:], in_=wg_raw[:])
            wf_t = ppool.tile([P, C], FP32, tag="wf")
            nc.vector.tensor_copy(out=wf_t[:], in_=wf_raw[:])

            part_t = ppool.tile([P, N_TILES], FP32, tag="part")
            sums_t = ppool.tile([P, M_CHUNKS], FP32, tag="sums")
            y1_t = ppool.tile([P, 1], FP32, tag="y1")
            y2_t = ppool.tile([P, M_CHUNKS], FP32, tag="y2")

            # Pass 1: stream fp16 x in; row-sum each tile as it lands. All
            # reduction forms run at 1 elem/cycle/partition (TensorReduce has
            # no packed mode, and the accumulating TensorScalarPtr falls back
            # to 1x on HW despite the cost model's claim; GPSIMD supports
            # neither free-axis reduce nor the accum form), so the work is
            # split between ACT (copy with fp32 FSA accum, 1.2 GHz) and DVE
            # (reduce_sum, 0.96 GHz) with a greedy arrival-aware schedule —
            # combined 0.50 tile/us vs the 0.42 tile/us DMA stream.
            def row_sum(t, col, eng):
                if eng == "D":
                    nc.vector.reduce_sum(
                        out=part_t[:, col : col + 1], in_=t[:], axis=AX
                    )
                else:
                    nc.scalar.activation(
                        out=t[:],
                        in_=t[:],
                        func=mybir.ActivationFunctionType.Copy,
                        accum_out=part_t[:, col : col + 1],
                    )

            # Greedy engine choice simulated against the DMA arrival clock
            # (2.4 us/MiB): pick the engine that can start each tile soonest.
            flat_w = [w for ws in WIDTHS for w in ws]
            arrive = []
            tclk = 0.0
            for w in flat_w:
                tclk += 2.4 * (w / 4096.0)
                arrive.append(tclk)
            ready = {"A": 2.5, "D": 4.0}   # ACT table-load; DVE staging copies
            engs = []
            for w, ar in zip(flat_w, arrive):
                starts = {e: max(ar, ready[e]) for e in ("A", "D")}
                dur = {"A": (w + 352) / 1200.0, "D": (w + 150) / 960.0}
                e = min(starts, key=lambda e: (starts[e] + dur[e]))
                ready[e] = starts[e] + dur[e]
                engs.append(e)

            cached = {}          # (m, j) -> resident [P, w] fp16 tile
            p1 = qpool.tile([P, 1], FP32, tag="p1")
            col = 0
            for m in range(M_CHUNKS):
                lo = col
                off = 0
                for j, w in enumerate(WIDTHS[m]):
                    t = cpool.tile([P, w], FP16, tag=f"c{m}_{j}")
                    cached[(m, j)] = t
                    nc.sync.dma_start(
                        out=t[:], in_=x_d[m * P : (m + 1) * P, off : off + w]
                    )
                    row_sum(t, col, engs[col])
                    off += w
                    col += 1
                # Chunk m fully reduced -> combine partials and fold into the
                # y1 matmul accumulation immediately; only chunk 3's combine
                # and matmul remain on the critical path after the last tile.
                nc.vector.reduce_sum(
                    out=sums_t[:, m : m + 1],
                    in_=part_t[:, lo:col],
                    axis=AX,
                )
                nc.tensor.matmul(
                    p1[:],
                    wg_t[:, m, :],
                    sums_t[:, m : m + 1],
                    start=(m == 0),
                    stop=(m == M_CHUNKS - 1),
                )

            # y1 = relu6(p1)
            nc.vector.tensor_scalar(
                out=y1_t[:], in0=p1[:], scalar1=0.0, scalar2=6.0, op0=ALU.max, op1=ALU.min
            )

            # y2 = relu6(wf.T @ y1 + b2): one [128,1] column per channel chunk.
            p2 = qpool.tile([P, M_CHUNKS], FP32, tag="p2")
            for m in range(M_CHUNKS):
                nc.tensor.matmul(
                    p2[:, m : m + 1],
                    wf_t[:, m * P : (m + 1) * P],
                    y1_t[:],
                    start=True,
                    stop=True,
                )
            nc.vector.tensor_add(out=y2_t[:], in0=p2[:], in1=b2_t[:, :M_CHUNKS])
            nc.vector.tensor_scalar(
                out=y2_t[:], in0=y2_t[:], scalar1=0.0, scalar2=6.0, op0=ALU.max, op1=ALU.min
            )

            # Pass 2: out = x + y2[channel], in place on the resident fp16
            # tiles (DVE 4x mode), then store on the SP ring. DVE outpaces the
            # store drain, so this pass is DMA-bound end to end.
            for m in range(M_CHUNKS):
                off = 0
                for j, w in enumerate(WIDTHS[m]):
                    t = cached[(m, j)]
                    nc.vector.tensor_scalar_add(
                        out=t[:], in0=t[:], scalar1=y2_t[:, m : m + 1]
                    )
                    nc.sync.dma_start(
                        out=out_d[m * P : (m + 1) * P, off : off + w], in_=t[:]
                    )
                    off += w

    _hoist_excess_waits(nc)
    return nc


# walrus codegen has per-instruction sync-wait slot limits (the Matmult
# LDWEIGHTS struct fits one wait; the DMA DIRECT2D struct fits two). Tile's
# sem assignment is not transitively minimal and can exceed them. Excess waits
# are hoisted into standalone EventSemaphore instructions placed right before
# the instruction on the same engine queue — identical semantics (inline DMA
# waits execute at the issuing sequencer too), just a different encoding.
_WAIT_CAPS = {
    "InstMatmult": 1,
    "InstActivation": 1,
    "InstDMACopy": 1,
    "InstTensorReduce": 1,
    "InstTensorScalarPtr": 1,
    "InstTensorTensor": 1,
    "InstTensorCopy": 1,
    "InstMemset": 1,
    "InstDrain": 1,
}


def _hoist_excess_waits(nc: bass.Bass) -> None:
    n = 0
    for bb in nc.main_func.blocks:
        il = bb.instructions
        new_list = []
        for ins in il:
            si = ins.sync_info
            cap = _WAIT_CAPS.get(type(ins).__name__)
            if si is not None and cap is not None and len(si.on_wait) > cap:
                waits = list(si.on_wait)
                for w in waits[cap:]:
                    n += 1
                    es = mybir.InstEventSemaphore(
                        name=f"I-hoistwait-{n}",
                        engine=ins.engine,
                        sync_info=mybir.SyncInfo(on_wait=[w], on_update=[]),
                    )
                    new_list.append(es)
                ins.sync_info = mybir.SyncInfo(
                    on_wait=waits[:cap], on_update=list(si.on_update)
                )
            new_list.append(ins)
        if len(new_list) != len(il):
            il[:] = new_list


_NC = None


def _get_nc() -> bass.Bass:
    global _NC
    if _NC is None:
        _NC = _build_program()
    return _NC


def _prep_in_maps(x, w_guide, w_fuse, bn_gamma, bn_beta, bn_mean, bn_var):
    x = np.asarray(x, dtype=np.float32)
    w_guide = np.asarray(w_guide, dtype=np.float32)
    w_fuse = np.asarray(w_fuse, dtype=np.float32)
    bn_gamma = np.asarray(bn_gamma, dtype=np.float32)
    bn_beta = np.asarray(bn_beta, dtype=np.float32)
    bn_mean = np.asarray(bn_mean, dtype=np.float32)
    bn_var = np.asarray(bn_var, dtype=np.float32)

    scale = bn_gamma / np.sqrt(bn_var + np.float32(BN_EPS))
    wg = np.ascontiguousarray((w_guide / np.float32(HW)).T)           # [C, R]
    wf = np.ascontiguousarray((w_fuse * scale[:, None]).T)            # [R, C]
    b2 = np.zeros((P, 128), dtype=np.float32)  # padded to 512 B DMA lines
    b2[:, :M_CHUNKS] = (bn_beta - bn_mean * scale).reshape(M_CHUNKS, P).T

    xs = np.ascontiguousarray(x.reshape(B, C, HW).astype(np.float16))
    return [{"x": xs[i], "wg": wg, "wf": wf, "b2": b2} for i in range(B)]


def run(inputs: dict, **kwargs):
    """Run the SPMD kernel; returns the BassKernelResults (for profiling)."""
    nc = _get_nc()
    in_maps = _prep_in_maps(**inputs)
    return run_bass_kernel_spmd(nc, in_maps, core_ids=list(range(B)), **kwargs)


def kernel(**inputs) -> np.ndarray:
    res = run(inputs)
    out = np.stack([np.asarray(res.results[i]["out"]) for i in range(B)], axis=0)
    return out.reshape(B, C, H, W).astype(np.float32)
